# revision 22
# baseline (speedup 1.0000x reference)
"""KnowledgeAwareAttention Trainium2 kernel (8-core SPMD, row-sharded).

attn[i,j] = sum_d R_emb[q[i,j],d] * x[j,d] * x[i,d]
out = softmax(attn, -1) @ x

Strategy per core (128 output rows), bf16 pipeline:
  - Host precomputes the 42 relation lhsT planes W_k = (x_I * R_k)^T in
    bf16 (two 128-d chunks each), so the PE just streams LDWEIGHTS+MATMUL
    (bf16 = 1 col/cycle, ~4x faster than the fp32 baseline; no on-chip
    lhs prep - the old GpSimd tensor_scalar_mul path cost 212us alone).
  - ~18 junk matmuls on a zeroed tile pre-warm the PE during the DMA
    lead-in (HAM clock gate: 1.2 -> 2.4 GHz after ~3.4us of activity).
  - Per pair (2m, 2m+1): 8 matmuls (2 planes x 2 d-chunks x 2 col-halves)
    into one 4-bank PSUM tile [128, 2048] f32, double-buffered. Pair 0
    skips relation 0 (zeroed padding row => T_0 == 0).
  - One wide ScalarE copy drains each pair PSUM->SBUF as bf16, then
    VectorE copy_predicated (uint16 bit-0 mask) does the level-0 mux
    merge in SBUF. Upper tree levels (bits 1..5) are emitted
    binary-counter style, interleaved into the pair loop so the FIFO
    VectorE queue overlaps them with phase B instead of serializing
    them at the end. VectorE runs ~41 merges back-to-back at ~1.13us
    each (copy_predicated has no 2x microcode mode) - this stream is
    the kernel's critical path.
  - softmax without max-subtraction (|attn| < ~0.2), exp on ScalarE with
    fused row-sum, reciprocal on VectorE.
  - output matmul: 8 PE transposes (bf16) + VectorE PSUM->SBUF copies +
    8 accumulating bf16 matmuls vs x chunks; final row-scale by 1/Z
    fused into the PSUM->SBUF copy on ScalarE.
"""

import numpy as np
import ml_dtypes

import concourse.bass as bass
import concourse.mybir as mybir
import concourse.tile as tile
from concourse.bass_utils import run_bass_kernel_spmd
from concourse.masks import make_identity

B = 1024
D = 256
NREL = 42
NCORES = 8
P = 128  # rows per core
NPAIR = NREL // 2  # 21
F32 = mybir.dt.float32
BF16 = mybir.dt.bfloat16
AF = mybir.ActivationFunctionType
NPBF16 = ml_dtypes.bfloat16

# w layout: per pair m, 4 blocks of 128 cols: (k=2m,c=0),(2m,1),(2m+1,0),(2m+1,1)
W_COLS = NREL * 2 * P  # 10752
# DMA chunking: tiny first chunk so the first matmul can start ASAP
W_CHUNK_PAIRS = [1, 3, 3, 3, 3, 3, 3, 2]
W_CHUNK_START = [sum(W_CHUNK_PAIRS[:i]) for i in range(len(W_CHUNK_PAIRS))]


def _patch_tile_tail_drain():
    """This container's walrus rejects >1 sync-wait command on the
    kernel-tail SP Drain. Split the waits across SP nops."""
    import concourse.mybir as mybir_
    import concourse.tile as tile_

    def _drain_and_barrier(self, tick_clock, wait_clock):
        nc = self.nc
        drain_inst = nc.sync.drain()
        wait_clock.add_sem_waits(
            drain_inst.ins, tile_.ScopedClock({None: tick_clock.global_clock})
        )
        si = drain_inst.ins.sync_info
        waits = list(si.on_wait) if si and si.on_wait else []
        if len(waits) > 1:
            si.on_wait = waits[:1]
            for w in waits[1:]:
                nop = nc.sync.nop(nofuse=True)
                nop.ins.sync_info = mybir_.SyncInfo(on_wait=[w], on_update=[])
        nc.all_engine_barrier()
        assert self.sems is not None
        popped = nc._tile_sem_poison_stack.pop()
        assert popped is self._sem_poison
        nc.clear_and_free_semaphores(list(self.sems.allocated().values()))
        nc.all_engine_barrier()

    tile_.TileContext._drain_and_barrier = _drain_and_barrier


_patch_tile_tail_drain()


_MAX_WAITS = 1


def _split_excess_waits(nc: bass.Bass, max_waits: int = _MAX_WAITS) -> None:
    """This container's walrus caps the number of sync-wait commands one
    instruction may carry. Move excess waits onto same-engine NoOps
    inserted immediately before the instruction."""
    cnt = 0
    for wrapper in nc.bb_map.values():
        bb = wrapper.bb
        old = list(bb.instructions)
        new = []
        changed = False
        for ins in old:
            si = ins.sync_info
            waits = list(si.on_wait) if si and si.on_wait else []
            if len(waits) > max_waits:
                changed = True
                si.on_wait = waits[:max_waits]
                rest = waits[max_waits:]
                for i in range(0, len(rest), max_waits):
                    nop = mybir.InstNoOp(name=f"waitnop{cnt}", ins=[], outs=[])
                    cnt += 1
                    nop.engine = ins.engine
                    nop.sync_info = mybir.SyncInfo(
                        on_wait=rest[i:i + max_waits], on_update=[]
                    )
                    new.append(nop)
            new.append(ins)
        if changed:
            bb.instructions = new


def build_nc() -> bass.Bass:
    nc = bass.Bass()
    xT_d = nc.dram_tensor("xt", [P, 2 * B], BF16, kind="ExternalInput")
    w_d = nc.dram_tensor("w", [P, W_COLS], BF16, kind="ExternalInput")
    xc_d = nc.dram_tensor("x", [P, 8 * D], BF16, kind="ExternalInput")
    bits_d = nc.dram_tensor("bits", [P, 6 * B], mybir.dt.uint16, kind="ExternalInput")
    out_d = nc.dram_tensor("out", [P, D], F32, kind="ExternalOutput")

    with tile.TileContext(nc) as tc:
        with (
            tc.tile_pool(name="const", bufs=1) as const,
            tc.tile_pool(name="planes", bufs=1) as planep,
            tc.tile_pool(name="sm", bufs=1) as smp,
            tc.tile_pool(name="et", bufs=4) as etp,
        ):
            # ---- loads (chunked so consumers start as data lands) ----
            # Issue order matters: the Sync queue serializes dma_starts at
            # ~0.7us each, and the first matmul gates on w chunk 0 + xT.
            xT_ts = [
                const.tile([P, B], BF16, tag=f"xt{c}", name=f"xt_t{c}")
                for c in range(2)
            ]
            w_ts = [
                const.tile([P, np_ * 4 * P], BF16, tag=f"w{ch}", name=f"w_t{ch}")
                for ch, np_ in enumerate(W_CHUNK_PAIRS)
            ]
            bit_ts = [
                const.tile([P, B], mybir.dt.uint16, tag=f"bit{l}", name=f"bit_t{l}")
                for l in range(6)
            ]

            def wdma(ch):
                lo = W_CHUNK_START[ch] * 4 * P
                nc.sync.dma_start(w_ts[ch][:, :],
                                  w_d[:, lo:lo + W_CHUNK_PAIRS[ch] * 4 * P])

            wdma(0)
            nc.sync.dma_start(xT_ts[0][:, :], xT_d[:, 0:B])
            nc.sync.dma_start(xT_ts[1][:, :], xT_d[:, B:2 * B])
            wdma(1)
            nc.sync.dma_start(bit_ts[0][:, :], bits_d[:, 0:B])
            for ch in range(2, len(W_CHUNK_PAIRS)):
                wdma(ch)
            for l in range(1, 6):
                nc.sync.dma_start(bit_ts[l][:, :], bits_d[:, l * B:(l + 1) * B])
            xc_t = const.tile([P, 8 * D], BF16, tag="xc", name="xc_t")
            nc.sync.dma_start(xc_t[:, :], xc_d[:, :])
            xT = [xT_ts[c][:, :] for c in range(2)]
            bits = [bit_ts[l][:, :] for l in range(6)]
            xc = [xc_t[:, j * D:(j + 1) * D] for j in range(8)]
            ident = const.tile([P, P], BF16, tag="ident")
            make_identity(nc, ident[:, :])
            warm = const.tile([P, P], BF16, tag="warm", name="warm_t")
            nc.gpsimd.memset(warm[:, :], 0.0)

            # ---- phase B: 21 plane-pairs, streaming mux-tree reduction ----
            # Per pair: 8 bf16 matmuls -> one wide ACT copy drains the 4-bank
            # PSUM tile to SBUF bf16 (frees PSUM fast; PE never waits on DVE),
            # then DVE does the level-0 copy_predicated merge in SBUF.
            # Upper tree levels are emitted binary-counter style, as soon as
            # both inputs exist, so the DVE queue (FIFO!) interleaves them
            # with level-0 merges instead of serializing them at the end.
            # Pair 0: relation 0 is the zeroed padding row (T_0 == 0), so skip
            # its matmuls and merge T_1 into a memset plane.
            stack = []  # (next_level, ap)

            def reduce_stack(collapse=False):
                while len(stack) >= 2:
                    (la, a), (lb, b) = stack[-2], stack[-1]
                    if la != lb and not collapse:
                        break
                    nc.vector.copy_predicated(a, bits[max(la, lb)], b)
                    stack.pop()
                    stack[-1] = (max(la, lb) + 1, a)

            with tc.tile_pool(name="pp", bufs=2, space="PSUM") as pp:
                for m in range(NPAIR):
                    pt = pp.tile([P, 2 * B], F32, tag="pair", name=f"t{m}")
                    if m == 0:
                        # PE warm-up during the DMA wait: ~30 junk matmuls
                        # release the HAM clock gate (1.2 -> 2.4 GHz) before
                        # the first real matmul arrives.
                        for wu in range(18):
                            nc.tensor.matmul(
                                pt[:, 0:P], lhsT=warm[:, :], rhs=warm[:, :],
                                start=True, stop=True,
                            )
                    ch = max(i for i, s in enumerate(W_CHUNK_START) if s <= m)
                    wch, wm = w_ts[ch], m - W_CHUNK_START[ch]
                    for parity in (0, 1):
                        if m == 0 and parity == 0:
                            continue
                        for c in range(2):
                            wblk = wch[:, (wm * 4 + parity * 2 + c) * P:
                                       (wm * 4 + parity * 2 + c + 1) * P]
                            for jh in range(2):
                                nc.tensor.matmul(
                                    pt[:, parity * B + jh * 512:
                                       parity * B + (jh + 1) * 512],
                                    lhsT=wblk,
                                    rhs=xT[c][:, jh * 512:(jh + 1) * 512],
                                    start=(c == 0),
                                    stop=(c == 1),
                                )
                    if m == 0:
                        S = planep.tile([P, 2 * B], BF16, tag="s0", name="s0")
                        nc.gpsimd.memset(S[:, 0:B], 0.0)
                        nc.scalar.copy(S[:, B:2 * B], pt[:, B:2 * B])
                    else:
                        S = planep.tile([P, 2 * B], BF16, tag=f"s{m}", name=f"s{m}")
                        nc.scalar.copy(S[:, :], pt[:, :])
                    nc.vector.copy_predicated(S[:, 0:B], bits[0], S[:, B:2 * B])
                    stack.append((1, S[:, 0:B]))
                    reduce_stack()
            reduce_stack(collapse=True)
            attn = stack[0][1]

            # ---- phase D: exp + row sums ----
            E = smp.tile([P, B], BF16, tag="E")
            z = smp.tile([P, 1], F32, tag="z")
            rz = smp.tile([P, 1], F32, tag="rz")
            nc.scalar.activation(E[:, :], attn, AF.Exp, accum_out=z[:, :])
            nc.vector.reciprocal(rz[:, :], z[:, :])

            # ---- phase E: transposes + output matmul ----
            with (
                tc.tile_pool(name="tp", bufs=4, space="PSUM") as tp,
                tc.tile_pool(name="op", bufs=1, space="PSUM") as op,
            ):
                out_ps = op.tile([P, D], F32, tag="out")
                for jc in range(8):
                    ptile = tp.tile([P, P], BF16, tag="tp", name=f"tp{jc}")
                    nc.tensor.transpose(ptile[:, :], E[:, jc * P:(jc + 1) * P], ident[:, :])
                    et = etp.tile([P, P], BF16, tag="et", name=f"et{jc}")
                    # DVE is idle by now; bf16 PSUM source gets 2x mode
                    nc.vector.tensor_copy(et[:, :], ptile[:, :])
                    nc.tensor.matmul(
                        out_ps[:, :],
                        lhsT=et[:, :],
                        rhs=xc[jc],
                        start=(jc == 0),
                        stop=(jc == 7),
                    )
                # ---- phase F: scale rows by 1/Z and store ----
                out_sb = smp.tile([P, D], F32, tag="osb")
                nc.scalar.activation(out_sb[:, :], out_ps[:, :], AF.Copy, scale=rz[:, :])
                nc.sync.dma_start(out_d[:, :], out_sb[:, :])
    _split_excess_waits(nc)
    return nc


_NC_CACHE = None


def _get_nc():
    global _NC_CACHE
    if _NC_CACHE is None:
        _NC_CACHE = build_nc()
    return _NC_CACHE


def make_in_maps(x, q, R):
    x = np.asarray(x, dtype=np.float32)
    q = np.asarray(q)
    R = np.asarray(R, dtype=np.float32)

    xT = np.ascontiguousarray(x.T)                      # [D, B]
    q32 = q.astype(np.int32)

    # packed [128, W] layouts: d-chunks side by side along the free axis
    xt_p = np.ascontiguousarray(
        xT.reshape(2, P, B).transpose(1, 0, 2).reshape(P, 2 * B)).astype(NPBF16)
    x_p = np.ascontiguousarray(
        x.reshape(8, P, D).transpose(1, 0, 2).reshape(P, 8 * D)).astype(NPBF16)

    in_maps = []
    for core in range(NCORES):
        rows = slice(core * P, (core + 1) * P)
        qb = q32[rows]                                   # [128, B]
        bits = np.empty((P, 6 * B), dtype=np.uint16)
        for l in range(6):
            bits[:, l * B:(l + 1) * B] = ((qb >> l) & 1).astype(np.uint16)
        # lhsT planes: w3[k, d, i] = R[k, d] * x[rows][i, d]
        xiT = x[rows].T                                  # [D, 128]
        w3 = R[:, :, None] * xiT[None, :, :]             # [42, 256, 128]
        # -> [42, 2, 128(d_lo), 128(i)] -> [128(d_lo), 21(pair), 2(par), 2(c), 128(i)]
        w5 = w3.reshape(NPAIR, 2, 2, P, P).transpose(3, 0, 1, 2, 4)
        w_p = np.ascontiguousarray(w5.reshape(P, W_COLS)).astype(NPBF16)
        in_maps.append(
            {
                "xt": xt_p,
                "w": w_p,
                "x": x_p,
                "bits": bits,
            }
        )
    return in_maps


def kernel(x, x_mask, q, f, R_emb):
    in_maps = make_in_maps(x, q, R_emb)
    res = run_bass_kernel_spmd(_get_nc(), in_maps, core_ids=list(range(NCORES)))
    out = np.concatenate([res.results[c]["out"] for c in range(NCORES)], axis=0)
    return out


# revision 23
# speedup vs baseline: 1.1765x; 1.1765x over previous
"""KnowledgeAwareAttention Trainium2 kernel (8-core SPMD, row-sharded).

attn[i,j] = sum_d R_emb[q[i,j],d] * x[j,d] * x[i,d]
out = softmax(attn, -1) @ x

Strategy per core (128 output rows), v2 (bf16 pipeline):
  - Host precomputes the 42 relation lhsT planes W_k = (x_I * R_k)^T in
    bf16 (two 128-d chunks each), so the PE just streams LDWEIGHTS+MATMUL
    (bf16 = 1 col/cycle, 4x faster than fp32; no on-chip lhs prep).
  - Per pair (2m, 2m+1): 8 matmuls (2 planes x 2 d-chunks x 2 col-halves)
    into one 4-bank PSUM tile [128, 2048] f32, double-buffered.
  - One wide ScalarE copy drains the pair PSUM->SBUF as bf16, then a
    VectorE copy_predicated with the host bf16 bit-0 mask does the
    level-0 mux merge. Upper tree levels (bits 1..5) are 20 more
    SBUF bf16 copy_predicated merges.
  - softmax without max-subtraction (|attn| < ~0.2), exp on ScalarE with
    fused row-sum, reciprocal on VectorE.
  - output matmul: 8 PE transposes (bf16) + 8 accumulating bf16 matmuls
    vs x chunks; final row-scale by 1/Z fused into the PSUM->SBUF copy.
"""

import numpy as np
import ml_dtypes

import concourse.bass as bass
import concourse.mybir as mybir
import concourse.tile as tile
from concourse.bass_utils import run_bass_kernel_spmd
from concourse.masks import make_identity

B = 1024
D = 256
NREL = 42
NCORES = 8
P = 128  # rows per core
NPAIR = NREL // 2  # 21
F32 = mybir.dt.float32
BF16 = mybir.dt.bfloat16
AF = mybir.ActivationFunctionType
NPBF16 = ml_dtypes.bfloat16

# w layout: per pair m, 4 blocks of 128 cols: (k=2m,c=0),(2m,1),(2m+1,0),(2m+1,1)
W_COLS = NREL * 2 * P  # 10752
# DMA chunking: tiny first chunk so the first matmul can start ASAP
W_CHUNK_PAIRS = [1, 3, 3, 3, 3, 3, 3, 2]
W_CHUNK_START = [sum(W_CHUNK_PAIRS[:i]) for i in range(len(W_CHUNK_PAIRS))]


def _patch_tile_tail_drain():
    """This container's walrus rejects >1 sync-wait command on the
    kernel-tail SP Drain. Split the waits across SP nops."""
    import concourse.mybir as mybir_
    import concourse.tile as tile_

    def _drain_and_barrier(self, tick_clock, wait_clock):
        nc = self.nc
        drain_inst = nc.sync.drain()
        wait_clock.add_sem_waits(
            drain_inst.ins, tile_.ScopedClock({None: tick_clock.global_clock})
        )
        si = drain_inst.ins.sync_info
        waits = list(si.on_wait) if si and si.on_wait else []
        if len(waits) > 1:
            si.on_wait = waits[:1]
            for w in waits[1:]:
                nop = nc.sync.nop(nofuse=True)
                nop.ins.sync_info = mybir_.SyncInfo(on_wait=[w], on_update=[])
        nc.all_engine_barrier()
        assert self.sems is not None
        popped = nc._tile_sem_poison_stack.pop()
        assert popped is self._sem_poison
        nc.clear_and_free_semaphores(list(self.sems.allocated().values()))
        nc.all_engine_barrier()

    tile_.TileContext._drain_and_barrier = _drain_and_barrier


_patch_tile_tail_drain()


_MAX_WAITS = 1


def _split_excess_waits(nc: bass.Bass, max_waits: int = _MAX_WAITS) -> None:
    """This container's walrus caps the number of sync-wait commands one
    instruction may carry. Move excess waits onto same-engine NoOps
    inserted immediately before the instruction."""
    cnt = 0
    for wrapper in nc.bb_map.values():
        bb = wrapper.bb
        old = list(bb.instructions)
        new = []
        changed = False
        for ins in old:
            si = ins.sync_info
            waits = list(si.on_wait) if si and si.on_wait else []
            if len(waits) > max_waits:
                changed = True
                si.on_wait = waits[:max_waits]
                rest = waits[max_waits:]
                for i in range(0, len(rest), max_waits):
                    nop = mybir.InstNoOp(name=f"waitnop{cnt}", ins=[], outs=[])
                    cnt += 1
                    nop.engine = ins.engine
                    nop.sync_info = mybir.SyncInfo(
                        on_wait=rest[i:i + max_waits], on_update=[]
                    )
                    new.append(nop)
            new.append(ins)
        if changed:
            bb.instructions = new


def build_nc() -> bass.Bass:
    nc = bass.Bass()
    xT_d = nc.dram_tensor("xt", [P, 2 * B], BF16, kind="ExternalInput")
    w_d = nc.dram_tensor("w", [P, W_COLS], BF16, kind="ExternalInput")
    xc_d = nc.dram_tensor("x", [P, 8 * D], BF16, kind="ExternalInput")
    bits_d = nc.dram_tensor("bits", [P, 6 * B], mybir.dt.uint16, kind="ExternalInput")
    out_d = nc.dram_tensor("out", [P, D], F32, kind="ExternalOutput")

    with tile.TileContext(nc) as tc:
        with (
            tc.tile_pool(name="const", bufs=1) as const,
            tc.tile_pool(name="planes", bufs=1) as planep,
            tc.tile_pool(name="sm", bufs=1) as smp,
            tc.tile_pool(name="et", bufs=4) as etp,
        ):
            # ---- loads (chunked so consumers start as data lands) ----
            # Issue order matters: the Sync queue serializes dma_starts at
            # ~0.7us each, and the first matmul gates on w chunk 0 + xT.
            xT_ts = [
                const.tile([P, B], BF16, tag=f"xt{c}", name=f"xt_t{c}")
                for c in range(2)
            ]
            w_ts = [
                const.tile([P, np_ * 4 * P], BF16, tag=f"w{ch}", name=f"w_t{ch}")
                for ch, np_ in enumerate(W_CHUNK_PAIRS)
            ]
            bit_ts = [
                const.tile([P, B], mybir.dt.uint16, tag=f"bit{l}", name=f"bit_t{l}")
                for l in range(6)
            ]

            def wdma(ch):
                lo = W_CHUNK_START[ch] * 4 * P
                nc.sync.dma_start(w_ts[ch][:, :],
                                  w_d[:, lo:lo + W_CHUNK_PAIRS[ch] * 4 * P])

            wdma(0)
            nc.sync.dma_start(xT_ts[0][:, :], xT_d[:, 0:B])
            nc.sync.dma_start(xT_ts[1][:, :], xT_d[:, B:2 * B])
            wdma(1)
            nc.sync.dma_start(bit_ts[0][:, :], bits_d[:, 0:B])
            for ch in range(2, len(W_CHUNK_PAIRS)):
                wdma(ch)
            for l in range(1, 6):
                nc.sync.dma_start(bit_ts[l][:, :], bits_d[:, l * B:(l + 1) * B])
            xc_t = const.tile([P, 8 * D], BF16, tag="xc", name="xc_t")
            nc.sync.dma_start(xc_t[:, :], xc_d[:, :])
            xT = [xT_ts[c][:, :] for c in range(2)]
            bits = [bit_ts[l][:, :] for l in range(6)]
            xc = [xc_t[:, j * D:(j + 1) * D] for j in range(8)]
            ident = const.tile([P, P], BF16, tag="ident")
            make_identity(nc, ident[:, :])
            warm = const.tile([P, P], BF16, tag="warm", name="warm_t")
            nc.gpsimd.memset(warm[:, :], 0.0)

            # ---- phase B: 21 plane-pairs, streaming mux-tree reduction ----
            # Per pair: 8 bf16 matmuls -> one wide ACT copy drains the 4-bank
            # PSUM tile to SBUF bf16 (frees PSUM fast; PE never waits on DVE),
            # then DVE does the level-0 copy_predicated merge in SBUF.
            # Upper tree levels are emitted binary-counter style, as soon as
            # both inputs exist, so the DVE queue (FIFO!) interleaves them
            # with level-0 merges instead of serializing them at the end.
            # Pair 0: relation 0 is the zeroed padding row (T_0 == 0), so skip
            # its matmuls and merge T_1 into a memset plane.
            stack = []  # (next_level, ap)

            def reduce_stack(collapse=False):
                while len(stack) >= 2:
                    (la, a), (lb, b) = stack[-2], stack[-1]
                    if la != lb and not collapse:
                        break
                    nc.vector.copy_predicated(a, bits[max(la, lb)], b)
                    stack.pop()
                    stack[-1] = (max(la, lb) + 1, a)

            with tc.tile_pool(name="pp", bufs=2, space="PSUM") as pp:
                for m in range(NPAIR):
                    pt = pp.tile([P, 2 * B], F32, tag="pair", name=f"t{m}")
                    if m == 0:
                        # PE warm-up during the DMA wait: ~30 junk matmuls
                        # release the HAM clock gate (1.2 -> 2.4 GHz) before
                        # the first real matmul arrives.
                        for wu in range(18):
                            nc.tensor.matmul(
                                pt[:, 0:P], lhsT=warm[:, :], rhs=warm[:, :],
                                start=True, stop=True,
                            )
                    ch = max(i for i, s in enumerate(W_CHUNK_START) if s <= m)
                    wch, wm = w_ts[ch], m - W_CHUNK_START[ch]
                    for parity in (0, 1):
                        if m == 0 and parity == 0:
                            continue
                        for c in range(2):
                            wblk = wch[:, (wm * 4 + parity * 2 + c) * P:
                                       (wm * 4 + parity * 2 + c + 1) * P]
                            for jh in range(2):
                                nc.tensor.matmul(
                                    pt[:, parity * B + jh * 512:
                                       parity * B + (jh + 1) * 512],
                                    lhsT=wblk,
                                    rhs=xT[c][:, jh * 512:(jh + 1) * 512],
                                    start=(c == 0),
                                    stop=(c == 1),
                                )
                    if m == 0:
                        S = planep.tile([P, 2 * B], BF16, tag="s0", name="s0")
                        nc.gpsimd.memset(S[:, 0:B], 0.0)
                        nc.scalar.copy(S[:, B:2 * B], pt[:, B:2 * B])
                    else:
                        S = planep.tile([P, 2 * B], BF16, tag=f"s{m}", name=f"s{m}")
                        nc.scalar.copy(S[:, :], pt[:, :])
                    nc.vector.copy_predicated(S[:, 0:B], bits[0], S[:, B:2 * B])
                    stack.append((1, S[:, 0:B]))
                    reduce_stack()
            reduce_stack(collapse=True)
            attn = stack[0][1]

            # ---- phase D: exp + row sums ----
            E = smp.tile([P, B], BF16, tag="E")
            z = smp.tile([P, 1], F32, tag="z")
            rz = smp.tile([P, 1], F32, tag="rz")
            nc.scalar.activation(E[:, :], attn, AF.Exp, accum_out=z[:, :])
            nc.vector.reciprocal(rz[:, :], z[:, :])

            # ---- phase E: transposes + output matmul ----
            with (
                tc.tile_pool(name="tp", bufs=4, space="PSUM") as tp,
                tc.tile_pool(name="op", bufs=1, space="PSUM") as op,
            ):
                out_ps = op.tile([P, D], F32, tag="out")
                for jc in range(8):
                    ptile = tp.tile([P, P], BF16, tag="tp", name=f"tp{jc}")
                    nc.tensor.transpose(ptile[:, :], E[:, jc * P:(jc + 1) * P], ident[:, :])
                    et = etp.tile([P, P], BF16, tag="et", name=f"et{jc}")
                    # DVE is idle by now; bf16 PSUM source gets 2x mode
                    nc.vector.tensor_copy(et[:, :], ptile[:, :])
                    nc.tensor.matmul(
                        out_ps[:, :],
                        lhsT=et[:, :],
                        rhs=xc[jc],
                        start=(jc == 0),
                        stop=(jc == 7),
                    )
                # ---- phase F: scale rows by 1/Z and store ----
                out_sb = smp.tile([P, D], F32, tag="osb")
                nc.scalar.activation(out_sb[:, :], out_ps[:, :], AF.Copy, scale=rz[:, :])
                nc.sync.dma_start(out_d[:, :], out_sb[:, :])
    _split_excess_waits(nc)
    return nc


_NC_CACHE = None


def _get_nc():
    global _NC_CACHE
    if _NC_CACHE is None:
        _NC_CACHE = build_nc()
    return _NC_CACHE


def make_in_maps(x, q, R):
    x = np.asarray(x, dtype=np.float32)
    q = np.asarray(q)
    R = np.asarray(R, dtype=np.float32)

    xT = np.ascontiguousarray(x.T)                      # [D, B]
    q32 = q.astype(np.int32)

    # packed [128, W] layouts: d-chunks side by side along the free axis
    xt_p = np.ascontiguousarray(
        xT.reshape(2, P, B).transpose(1, 0, 2).reshape(P, 2 * B)).astype(NPBF16)
    x_p = np.ascontiguousarray(
        x.reshape(8, P, D).transpose(1, 0, 2).reshape(P, 8 * D)).astype(NPBF16)

    in_maps = []
    for core in range(NCORES):
        rows = slice(core * P, (core + 1) * P)
        qb = q32[rows]                                   # [128, B]
        bits = np.empty((P, 6 * B), dtype=np.uint16)
        for l in range(6):
            bits[:, l * B:(l + 1) * B] = ((qb >> l) & 1).astype(np.uint16)
        # lhsT planes: w3[k, d, i] = R[k, d] * x[rows][i, d]
        xiT = x[rows].T                                  # [D, 128]
        w3 = R[:, :, None] * xiT[None, :, :]             # [42, 256, 128]
        # -> [42, 2, 128(d_lo), 128(i)] -> [128(d_lo), 21(pair), 2(par), 2(c), 128(i)]
        w5 = w3.reshape(NPAIR, 2, 2, P, P).transpose(3, 0, 1, 2, 4)
        w_p = np.ascontiguousarray(w5.reshape(P, W_COLS)).astype(NPBF16)
        in_maps.append(
            {
                "xt": xt_p,
                "w": w_p,
                "x": x_p,
                "bits": bits,
            }
        )
    return in_maps


def kernel(x, x_mask, q, f, R_emb):
    in_maps = make_in_maps(x, q, R_emb)
    res = run_bass_kernel_spmd(_get_nc(), in_maps, core_ids=list(range(NCORES)))
    out = np.concatenate([res.results[c]["out"] for c in range(NCORES)], axis=0)
    return out


# revision 29
# speedup vs baseline: 1.2084x; 1.0271x over previous
"""KnowledgeAwareAttention Trainium2 kernel (8-core SPMD, row-sharded).

attn[i,j] = sum_d R_emb[q[i,j],d] * x[j,d] * x[i,d]
out = softmax(attn, -1) @ x

Strategy per core (128 output rows), v2 (bf16 pipeline):
  - Host precomputes the 42 relation lhsT planes W_k = (x_I * R_k)^T in
    bf16 (two 128-d chunks each), so the PE just streams LDWEIGHTS+MATMUL
    (bf16 = 1 col/cycle, 4x faster than fp32; no on-chip lhs prep).
  - Per pair (2m, 2m+1): 8 matmuls (2 planes x 2 d-chunks x 2 col-halves)
    into one 4-bank PSUM tile [128, 2048] f32, double-buffered.
  - One wide ScalarE copy drains the pair PSUM->SBUF as bf16, then a
    VectorE copy_predicated with the host bf16 bit-0 mask does the
    level-0 mux merge. Upper tree levels (bits 1..5) are 20 more
    SBUF bf16 copy_predicated merges.
  - softmax without max-subtraction (|attn| < ~0.2), exp on ScalarE with
    fused row-sum, reciprocal on VectorE.
  - output matmul: 8 PE transposes (bf16) + 8 accumulating bf16 matmuls
    vs x chunks; final row-scale by 1/Z fused into the PSUM->SBUF copy.
"""

import numpy as np
import ml_dtypes

import concourse.bass as bass
import concourse.mybir as mybir
import concourse.tile as tile
from concourse.bass_utils import run_bass_kernel_spmd
from concourse.masks import make_identity

B = 1024
D = 256
NREL = 42
NCORES = 8
P = 128  # rows per core
NPAIR = NREL // 2  # 21
F32 = mybir.dt.float32
BF16 = mybir.dt.bfloat16
AF = mybir.ActivationFunctionType
NPBF16 = ml_dtypes.bfloat16

# w layout: per pair m, 4 blocks of 128 cols: (k=2m,c=0),(2m,1),(2m+1,0),(2m+1,1)
W_COLS = NREL * 2 * P  # 10752
# DMA chunking: tiny first chunk so the first matmul can start ASAP
W_CHUNK_PAIRS = [1, 3, 3, 3, 3, 3, 3, 2]
W_CHUNK_START = [sum(W_CHUNK_PAIRS[:i]) for i in range(len(W_CHUNK_PAIRS))]


def _patch_tile_tail_drain():
    """This container's walrus rejects >1 sync-wait command on the
    kernel-tail SP Drain. Split the waits across SP nops."""
    import concourse.mybir as mybir_
    import concourse.tile as tile_

    def _drain_and_barrier(self, tick_clock, wait_clock):
        nc = self.nc
        drain_inst = nc.sync.drain()
        wait_clock.add_sem_waits(
            drain_inst.ins, tile_.ScopedClock({None: tick_clock.global_clock})
        )
        si = drain_inst.ins.sync_info
        waits = list(si.on_wait) if si and si.on_wait else []
        if len(waits) > 1:
            si.on_wait = waits[:1]
            for w in waits[1:]:
                nop = nc.sync.nop(nofuse=True)
                nop.ins.sync_info = mybir_.SyncInfo(on_wait=[w], on_update=[])
        nc.all_engine_barrier()
        assert self.sems is not None
        popped = nc._tile_sem_poison_stack.pop()
        assert popped is self._sem_poison
        nc.clear_and_free_semaphores(list(self.sems.allocated().values()))
        nc.all_engine_barrier()

    tile_.TileContext._drain_and_barrier = _drain_and_barrier


_patch_tile_tail_drain()


_MAX_WAITS = 1


def _split_excess_waits(nc: bass.Bass, max_waits: int = _MAX_WAITS) -> None:
    """This container's walrus caps the number of sync-wait commands one
    instruction may carry. Move excess waits onto same-engine NoOps
    inserted immediately before the instruction."""
    cnt = 0
    for wrapper in nc.bb_map.values():
        bb = wrapper.bb
        old = list(bb.instructions)
        new = []
        changed = False
        for ins in old:
            si = ins.sync_info
            waits = list(si.on_wait) if si and si.on_wait else []
            if len(waits) > max_waits:
                changed = True
                si.on_wait = waits[:max_waits]
                rest = waits[max_waits:]
                for i in range(0, len(rest), max_waits):
                    nop = mybir.InstNoOp(name=f"waitnop{cnt}", ins=[], outs=[])
                    cnt += 1
                    nop.engine = ins.engine
                    nop.sync_info = mybir.SyncInfo(
                        on_wait=rest[i:i + max_waits], on_update=[]
                    )
                    new.append(nop)
            new.append(ins)
        if changed:
            bb.instructions = new


def build_nc() -> bass.Bass:
    nc = bass.Bass()
    # wx = pair-0 weight blocks (512) + xT both chunks (2048): one DMA
    # carries everything the first matmuls need (each dma_start costs
    # ~0.65us of serialized issue time on the Sync queue).
    wx_d = nc.dram_tensor("wx", [P, 512 + 2 * B], BF16, kind="ExternalInput")
    w_d = nc.dram_tensor("w", [P, W_COLS], BF16, kind="ExternalInput")
    xc_d = nc.dram_tensor("x", [P, 8 * D], BF16, kind="ExternalInput")
    bits_d = nc.dram_tensor("bits", [P, 6 * B], mybir.dt.uint16, kind="ExternalInput")
    out_d = nc.dram_tensor("out", [P, D], F32, kind="ExternalOutput")

    with tile.TileContext(nc) as tc:
        with (
            tc.tile_pool(name="const", bufs=1) as const,
            tc.tile_pool(name="planes", bufs=1) as planep,
            tc.tile_pool(name="sm", bufs=1) as smp,
            tc.tile_pool(name="et", bufs=4) as etp,
        ):
            # ---- loads (chunked so consumers start as data lands) ----
            # Issue order matters: the Sync queue serializes dma_starts at
            # ~0.7us each, and the first matmul gates on the wx bundle.
            wx_t = const.tile([P, 512 + 2 * B], BF16, tag="wx", name="wx_t")
            w_ts = [None] + [
                const.tile([P, np_ * 4 * P], BF16, tag=f"w{ch}", name=f"w_t{ch}")
                for ch, np_ in enumerate(W_CHUNK_PAIRS[1:], start=1)
            ]
            bit0_t = const.tile([P, B], mybir.dt.uint16, tag="bit0", name="bit0_t")
            b15_t = const.tile([P, 5 * B], mybir.dt.uint16, tag="b15", name="b15_t")
            xc_t = const.tile([P, 8 * D], BF16, tag="xc", name="xc_t")

            def wdma(ch):
                lo = W_CHUNK_START[ch] * 4 * P
                nc.sync.dma_start(w_ts[ch][:, :],
                                  w_d[:, lo:lo + W_CHUNK_PAIRS[ch] * 4 * P])

            nc.sync.dma_start(wx_t[:, :], wx_d[:, :])
            wdma(1)
            nc.sync.dma_start(bit0_t[:, :], bits_d[:, 0:B])
            wdma(2)
            wdma(3)
            wdma(4)
            nc.sync.dma_start(b15_t[:, :], bits_d[:, B:6 * B])
            for ch in range(5, len(W_CHUNK_PAIRS)):
                wdma(ch)
            nc.sync.dma_start(xc_t[:, :], xc_d[:, :])
            xT = [wx_t[:, 512 + c * B:512 + (c + 1) * B] for c in range(2)]
            bits = [bit0_t[:, :]] + [b15_t[:, (l - 1) * B:l * B] for l in range(1, 6)]
            xc = [xc_t[:, j * D:(j + 1) * D] for j in range(8)]
            ident = const.tile([P, P], BF16, tag="ident")
            make_identity(nc, ident[:, :])
            warm = const.tile([P, P], BF16, tag="warm", name="warm_t")
            nc.gpsimd.memset(warm[:, :], 0.0)

            # ---- phase B: 21 plane-pairs, streaming mux-tree reduction ----
            # Per pair: 8 bf16 matmuls -> one wide ACT copy drains the 4-bank
            # PSUM tile to SBUF bf16 (frees PSUM fast; PE never waits on DVE),
            # then DVE does the level-0 copy_predicated merge in SBUF.
            # Upper tree levels are emitted binary-counter style, as soon as
            # both inputs exist, so the DVE queue (FIFO!) interleaves them
            # with level-0 merges instead of serializing them at the end.
            # Pair 0: relation 0 is the zeroed padding row (T_0 == 0), so skip
            # its matmuls and merge T_1 into a memset plane.
            stack = []  # (next_level, ap)

            def reduce_stack(collapse=False):
                while len(stack) >= 2:
                    (la, a), (lb, b) = stack[-2], stack[-1]
                    if la != lb and not collapse:
                        break
                    nc.vector.copy_predicated(a, bits[max(la, lb)], b)
                    stack.pop()
                    stack[-1] = (max(la, lb) + 1, a)

            with tc.tile_pool(name="pp", bufs=2, space="PSUM") as pp:
                for m in range(NPAIR):
                    pt = pp.tile([P, 2 * B], F32, tag="pair", name=f"t{m}")
                    if m == 0:
                        # PE warm-up during the DMA wait: ~30 junk matmuls
                        # release the HAM clock gate (1.2 -> 2.4 GHz) before
                        # the first real matmul arrives.
                        for wu in range(12):
                            nc.tensor.matmul(
                                pt[:, 0:P], lhsT=warm[:, :], rhs=warm[:, :],
                                start=True, stop=True,
                            )
                    if m == 0:
                        wtile, wbase = wx_t, 0
                    else:
                        ch = max(i for i, s in enumerate(W_CHUNK_START) if s <= m)
                        wtile, wbase = w_ts[ch], (m - W_CHUNK_START[ch]) * 4 * P
                    for parity in (0, 1):
                        if m == 0 and parity == 0:
                            continue
                        for c in range(2):
                            off = wbase + (parity * 2 + c) * P
                            wblk = wtile[:, off:off + P]
                            for jh in range(2):
                                nc.tensor.matmul(
                                    pt[:, parity * B + jh * 512:
                                       parity * B + (jh + 1) * 512],
                                    lhsT=wblk,
                                    rhs=xT[c][:, jh * 512:(jh + 1) * 512],
                                    start=(c == 0),
                                    stop=(c == 1),
                                )
                    if m == 0:
                        S = planep.tile([P, 2 * B], BF16, tag="s0", name="s0")
                        nc.gpsimd.memset(S[:, 0:B], 0.0)
                        nc.scalar.copy(S[:, B:2 * B], pt[:, B:2 * B])
                    else:
                        S = planep.tile([P, 2 * B], BF16, tag=f"s{m}", name=f"s{m}")
                        nc.scalar.copy(S[:, :], pt[:, :])
                    nc.vector.copy_predicated(S[:, 0:B], bits[0], S[:, B:2 * B])
                    stack.append((1, S[:, 0:B]))
                    reduce_stack()
            reduce_stack(collapse=True)
            attn = stack[0][1]

            # ---- phase D: exp + row sums ----
            E = smp.tile([P, B], BF16, tag="E")
            z = smp.tile([P, 1], F32, tag="z")
            rz = smp.tile([P, 1], F32, tag="rz")
            nc.scalar.activation(E[:, :], attn, AF.Exp, accum_out=z[:, :])
            nc.vector.reciprocal(rz[:, :], z[:, :])

            # ---- phase E: transposes + output matmul ----
            with (
                tc.tile_pool(name="tp", bufs=4, space="PSUM") as tp,
                tc.tile_pool(name="op", bufs=1, space="PSUM") as op,
            ):
                out_ps = op.tile([P, D], F32, tag="out")
                for jc in range(8):
                    ptile = tp.tile([P, P], BF16, tag="tp", name=f"tp{jc}")
                    nc.tensor.transpose(ptile[:, :], E[:, jc * P:(jc + 1) * P], ident[:, :])
                    et = etp.tile([P, P], BF16, tag="et", name=f"et{jc}")
                    # DVE is idle by now; bf16 PSUM source gets 2x mode
                    nc.vector.tensor_copy(et[:, :], ptile[:, :])
                    nc.tensor.matmul(
                        out_ps[:, :],
                        lhsT=et[:, :],
                        rhs=xc[jc],
                        start=(jc == 0),
                        stop=(jc == 7),
                    )
                # ---- phase F: scale rows by 1/Z and store ----
                out_sb = smp.tile([P, D], F32, tag="osb")
                nc.scalar.activation(out_sb[:, :], out_ps[:, :], AF.Copy, scale=rz[:, :])
                nc.sync.dma_start(out_d[:, :], out_sb[:, :])
    _split_excess_waits(nc)
    return nc


_NC_CACHE = None


def _get_nc():
    global _NC_CACHE
    if _NC_CACHE is None:
        _NC_CACHE = build_nc()
    return _NC_CACHE


def make_in_maps(x, q, R):
    x = np.asarray(x, dtype=np.float32)
    q = np.asarray(q)
    R = np.asarray(R, dtype=np.float32)

    xT = np.ascontiguousarray(x.T)                      # [D, B]
    q32 = q.astype(np.int32)

    # packed [128, W] layouts: d-chunks side by side along the free axis
    xt_p = np.ascontiguousarray(
        xT.reshape(2, P, B).transpose(1, 0, 2).reshape(P, 2 * B)).astype(NPBF16)
    x_p = np.ascontiguousarray(
        x.reshape(8, P, D).transpose(1, 0, 2).reshape(P, 8 * D)).astype(NPBF16)

    in_maps = []
    for core in range(NCORES):
        rows = slice(core * P, (core + 1) * P)
        qb = q32[rows]                                   # [128, B]
        bits = np.empty((P, 6 * B), dtype=np.uint16)
        for l in range(6):
            bits[:, l * B:(l + 1) * B] = ((qb >> l) & 1).astype(np.uint16)
        # lhsT planes: w3[k, d, i] = R[k, d] * x[rows][i, d]
        xiT = x[rows].T                                  # [D, 128]
        w3 = R[:, :, None] * xiT[None, :, :]             # [42, 256, 128]
        # -> [42, 2, 128(d_lo), 128(i)] -> [128(d_lo), 21(pair), 2(par), 2(c), 128(i)]
        w5 = w3.reshape(NPAIR, 2, 2, P, P).transpose(3, 0, 1, 2, 4)
        w_p = np.ascontiguousarray(w5.reshape(P, W_COLS)).astype(NPBF16)
        wx_p = np.ascontiguousarray(
            np.concatenate([w_p[:, 0:512], xt_p], axis=1))
        in_maps.append(
            {
                "wx": wx_p,
                "w": w_p,
                "x": x_p,
                "bits": bits,
            }
        )
    return in_maps


def kernel(x, x_mask, q, f, R_emb):
    in_maps = make_in_maps(x, q, R_emb)
    res = run_bass_kernel_spmd(_get_nc(), in_maps, core_ids=list(range(NCORES)))
    out = np.concatenate([res.results[c]["out"] for c in range(NCORES)], axis=0)
    return out


# revision 30
# speedup vs baseline: 1.2305x; 1.0183x over previous
"""KnowledgeAwareAttention Trainium2 kernel (8-core SPMD, row-sharded).

attn[i,j] = sum_d R_emb[q[i,j],d] * x[j,d] * x[i,d]
out = softmax(attn, -1) @ x

Strategy per core (128 output rows), v2 (bf16 pipeline):
  - Host precomputes the 42 relation lhsT planes W_k = (x_I * R_k)^T in
    bf16 (two 128-d chunks each), so the PE just streams LDWEIGHTS+MATMUL
    (bf16 = 1 col/cycle, 4x faster than fp32; no on-chip lhs prep).
  - Per pair (2m, 2m+1): 8 matmuls (2 planes x 2 d-chunks x 2 col-halves)
    into one 4-bank PSUM tile [128, 2048] f32, double-buffered.
  - One wide ScalarE copy drains the pair PSUM->SBUF as bf16, then a
    VectorE copy_predicated with the host bf16 bit-0 mask does the
    level-0 mux merge. Upper tree levels (bits 1..5) are 20 more
    SBUF bf16 copy_predicated merges.
  - softmax without max-subtraction (|attn| < ~0.2), exp on ScalarE with
    fused row-sum, reciprocal on VectorE.
  - output matmul: 8 PE transposes (bf16) + 8 accumulating bf16 matmuls
    vs x chunks; final row-scale by 1/Z fused into the PSUM->SBUF copy.
"""

import numpy as np
import ml_dtypes

import concourse.bass as bass
import concourse.mybir as mybir
import concourse.tile as tile
from concourse.bass_utils import run_bass_kernel_spmd
from concourse.masks import make_identity

B = 1024
D = 256
NREL = 42
NCORES = 8
P = 128  # rows per core
NPAIR = NREL // 2  # 21
F32 = mybir.dt.float32
BF16 = mybir.dt.bfloat16
AF = mybir.ActivationFunctionType
NPBF16 = ml_dtypes.bfloat16

# w layout: per pair m, 4 blocks of 128 cols: (k=2m,c=0),(2m,1),(2m+1,0),(2m+1,1)
W_COLS = NREL * 2 * P  # 10752
# DMA chunking: tiny first chunk so the first matmul can start ASAP
W_CHUNK_PAIRS = [1, 3, 3, 3, 3, 3, 3, 2]
W_CHUNK_START = [sum(W_CHUNK_PAIRS[:i]) for i in range(len(W_CHUNK_PAIRS))]


def _patch_tile_tail_drain():
    """This container's walrus rejects >1 sync-wait command on the
    kernel-tail SP Drain. Split the waits across SP nops."""
    import concourse.mybir as mybir_
    import concourse.tile as tile_

    def _drain_and_barrier(self, tick_clock, wait_clock):
        nc = self.nc
        drain_inst = nc.sync.drain()
        wait_clock.add_sem_waits(
            drain_inst.ins, tile_.ScopedClock({None: tick_clock.global_clock})
        )
        si = drain_inst.ins.sync_info
        waits = list(si.on_wait) if si and si.on_wait else []
        if len(waits) > 1:
            si.on_wait = waits[:1]
            for w in waits[1:]:
                nop = nc.sync.nop(nofuse=True)
                nop.ins.sync_info = mybir_.SyncInfo(on_wait=[w], on_update=[])
        nc.all_engine_barrier()
        assert self.sems is not None
        popped = nc._tile_sem_poison_stack.pop()
        assert popped is self._sem_poison
        nc.clear_and_free_semaphores(list(self.sems.allocated().values()))
        nc.all_engine_barrier()

    tile_.TileContext._drain_and_barrier = _drain_and_barrier


_patch_tile_tail_drain()


_MAX_WAITS = 1


def _split_excess_waits(nc: bass.Bass, max_waits: int = _MAX_WAITS) -> None:
    """This container's walrus caps the number of sync-wait commands one
    instruction may carry. Move excess waits onto same-engine NoOps
    inserted immediately before the instruction."""
    cnt = 0
    for wrapper in nc.bb_map.values():
        bb = wrapper.bb
        old = list(bb.instructions)
        new = []
        changed = False
        for ins in old:
            si = ins.sync_info
            waits = list(si.on_wait) if si and si.on_wait else []
            if len(waits) > max_waits:
                changed = True
                si.on_wait = waits[:max_waits]
                rest = waits[max_waits:]
                for i in range(0, len(rest), max_waits):
                    nop = mybir.InstNoOp(name=f"waitnop{cnt}", ins=[], outs=[])
                    cnt += 1
                    nop.engine = ins.engine
                    nop.sync_info = mybir.SyncInfo(
                        on_wait=rest[i:i + max_waits], on_update=[]
                    )
                    new.append(nop)
            new.append(ins)
        if changed:
            bb.instructions = new


def build_nc() -> bass.Bass:
    nc = bass.Bass()
    # wx = pair-0 weight blocks (512) + xT both chunks (2048): one DMA
    # carries everything the first matmuls need (each dma_start costs
    # ~0.65us of serialized issue time on the Sync queue).
    wx_d = nc.dram_tensor("wx", [P, 512 + 2 * B], BF16, kind="ExternalInput")
    w_d = nc.dram_tensor("w", [P, W_COLS], BF16, kind="ExternalInput")
    xc_d = nc.dram_tensor("x", [P, 8 * D], BF16, kind="ExternalInput")
    bits_d = nc.dram_tensor("bits", [P, 6 * B], mybir.dt.uint16, kind="ExternalInput")
    out_d = nc.dram_tensor("out", [P, D], F32, kind="ExternalOutput")

    with tile.TileContext(nc) as tc:
        with (
            tc.tile_pool(name="const", bufs=1) as const,
            tc.tile_pool(name="planes", bufs=1) as planep,
            tc.tile_pool(name="sm", bufs=1) as smp,
            tc.tile_pool(name="et", bufs=4) as etp,
        ):
            # ---- loads (chunked so consumers start as data lands) ----
            # Issue order matters: the Sync queue serializes dma_starts at
            # ~0.7us each, and the first matmul gates on the wx bundle.
            wx_t = const.tile([P, 512 + 2 * B], BF16, tag="wx", name="wx_t")
            w_ts = [None] + [
                const.tile([P, np_ * 4 * P], BF16, tag=f"w{ch}", name=f"w_t{ch}")
                for ch, np_ in enumerate(W_CHUNK_PAIRS[1:], start=1)
            ]
            bit0_t = const.tile([P, B], mybir.dt.uint16, tag="bit0", name="bit0_t")
            b15_t = const.tile([P, 5 * B], mybir.dt.uint16, tag="b15", name="b15_t")
            xc_t = const.tile([P, 8 * D], BF16, tag="xc", name="xc_t")

            def wdma(ch):
                lo = W_CHUNK_START[ch] * 4 * P
                nc.sync.dma_start(w_ts[ch][:, :],
                                  w_d[:, lo:lo + W_CHUNK_PAIRS[ch] * 4 * P])

            nc.sync.dma_start(wx_t[:, :], wx_d[:, :])
            wdma(1)
            nc.sync.dma_start(bit0_t[:, :], bits_d[:, 0:B])
            wdma(2)
            wdma(3)
            wdma(4)
            nc.sync.dma_start(b15_t[:, :], bits_d[:, B:6 * B])
            for ch in range(5, len(W_CHUNK_PAIRS)):
                wdma(ch)
            nc.sync.dma_start(xc_t[:, :], xc_d[:, :])
            xT = [wx_t[:, 512 + c * B:512 + (c + 1) * B] for c in range(2)]
            bits = [bit0_t[:, :]] + [b15_t[:, (l - 1) * B:l * B] for l in range(1, 6)]
            xc = [xc_t[:, j * D:(j + 1) * D] for j in range(8)]
            ident = const.tile([P, P], BF16, tag="ident")
            make_identity(nc, ident[:, :])
            warm = const.tile([P, P], BF16, tag="warm", name="warm_t")
            nc.gpsimd.memset(warm[:, :], 0.0)

            # ---- phase B: 21 plane-pairs, streaming mux-tree reduction ----
            # Per pair: 8 bf16 matmuls -> one wide ACT copy drains the 4-bank
            # PSUM tile to SBUF bf16 (frees PSUM fast; PE never waits on DVE),
            # then DVE does the level-0 copy_predicated merge in SBUF.
            # Upper tree levels are emitted binary-counter style, as soon as
            # both inputs exist, so the DVE queue (FIFO!) interleaves them
            # with level-0 merges instead of serializing them at the end.
            # Pair 0: relation 0 is the zeroed padding row (T_0 == 0), so skip
            # its matmuls and merge T_1 into a memset plane.
            stack = []  # (next_level, ap)

            def reduce_stack(collapse=False):
                while len(stack) >= 2:
                    (la, a), (lb, b) = stack[-2], stack[-1]
                    if la != lb and not collapse:
                        break
                    nc.vector.copy_predicated(a, bits[max(la, lb)], b)
                    stack.pop()
                    stack[-1] = (max(la, lb) + 1, a)

            with tc.tile_pool(name="pp", bufs=2, space="PSUM") as pp:
                for m in range(NPAIR):
                    pt = pp.tile([P, 2 * B], F32, tag="pair", name=f"t{m}")
                    if m == 0:
                        # PE warm-up during the DMA wait: ~30 junk matmuls
                        # release the HAM clock gate (1.2 -> 2.4 GHz) before
                        # the first real matmul arrives.
                        for wu in range(12):
                            nc.tensor.matmul(
                                pt[:, 0:P], lhsT=warm[:, :], rhs=warm[:, :],
                                start=True, stop=True,
                            )
                    if m == 0:
                        wtile, wbase = wx_t, 0
                    else:
                        ch = max(i for i, s in enumerate(W_CHUNK_START) if s <= m)
                        wtile, wbase = w_ts[ch], (m - W_CHUNK_START[ch]) * 4 * P
                    for parity in (0, 1):
                        if m == 0 and parity == 0:
                            continue
                        for c in range(2):
                            off = wbase + (parity * 2 + c) * P
                            wblk = wtile[:, off:off + P]
                            for jh in range(2):
                                nc.tensor.matmul(
                                    pt[:, parity * B + jh * 512:
                                       parity * B + (jh + 1) * 512],
                                    lhsT=wblk,
                                    rhs=xT[c][:, jh * 512:(jh + 1) * 512],
                                    start=(c == 0),
                                    stop=(c == 1),
                                )
                    if m == 0:
                        S = planep.tile([P, 2 * B], BF16, tag="s0", name="s0")
                        nc.gpsimd.memset(S[:, 0:B], 0.0)
                        nc.scalar.copy(S[:, B:2 * B], pt[:, B:2 * B])
                        nc.vector.copy_predicated(S[:, 0:B], bits[0], S[:, B:2 * B])
                    elif m <= 3:
                        # Early ramp: DVE is idle while pair production trickles
                        # in, so merge odd->even inside PSUM (fp32) instead of
                        # waiting on the wide ACT drain; ACT then copies only
                        # the merged half.
                        S = planep.tile([P, B], BF16, tag=f"s{m}", name=f"s{m}")
                        nc.vector.copy_predicated(pt[:, 0:B], bits[0], pt[:, B:2 * B])
                        nc.scalar.copy(S[:, :], pt[:, 0:B])
                    else:
                        S = planep.tile([P, 2 * B], BF16, tag=f"s{m}", name=f"s{m}")
                        nc.scalar.copy(S[:, :], pt[:, :])
                        nc.vector.copy_predicated(S[:, 0:B], bits[0], S[:, B:2 * B])
                    stack.append((1, S[:, 0:B]))
                    reduce_stack()
            reduce_stack(collapse=True)
            attn = stack[0][1]

            # ---- phase D: exp + row sums ----
            E = smp.tile([P, B], BF16, tag="E")
            z = smp.tile([P, 1], F32, tag="z")
            rz = smp.tile([P, 1], F32, tag="rz")
            nc.scalar.activation(E[:, :], attn, AF.Exp, accum_out=z[:, :])
            nc.vector.reciprocal(rz[:, :], z[:, :])

            # ---- phase E: transposes + output matmul ----
            with (
                tc.tile_pool(name="tp", bufs=4, space="PSUM") as tp,
                tc.tile_pool(name="op", bufs=1, space="PSUM") as op,
            ):
                out_ps = op.tile([P, D], F32, tag="out")
                for jc in range(8):
                    ptile = tp.tile([P, P], BF16, tag="tp", name=f"tp{jc}")
                    nc.tensor.transpose(ptile[:, :], E[:, jc * P:(jc + 1) * P], ident[:, :])
                    et = etp.tile([P, P], BF16, tag="et", name=f"et{jc}")
                    # DVE is idle by now; bf16 PSUM source gets 2x mode
                    nc.vector.tensor_copy(et[:, :], ptile[:, :])
                    nc.tensor.matmul(
                        out_ps[:, :],
                        lhsT=et[:, :],
                        rhs=xc[jc],
                        start=(jc == 0),
                        stop=(jc == 7),
                    )
                # ---- phase F: scale rows by 1/Z and store ----
                out_sb = smp.tile([P, D], F32, tag="osb")
                nc.scalar.activation(out_sb[:, :], out_ps[:, :], AF.Copy, scale=rz[:, :])
                nc.sync.dma_start(out_d[:, :], out_sb[:, :])
    _split_excess_waits(nc)
    return nc


_NC_CACHE = None


def _get_nc():
    global _NC_CACHE
    if _NC_CACHE is None:
        _NC_CACHE = build_nc()
    return _NC_CACHE


def make_in_maps(x, q, R):
    x = np.asarray(x, dtype=np.float32)
    q = np.asarray(q)
    R = np.asarray(R, dtype=np.float32)

    xT = np.ascontiguousarray(x.T)                      # [D, B]
    q32 = q.astype(np.int32)

    # packed [128, W] layouts: d-chunks side by side along the free axis
    xt_p = np.ascontiguousarray(
        xT.reshape(2, P, B).transpose(1, 0, 2).reshape(P, 2 * B)).astype(NPBF16)
    x_p = np.ascontiguousarray(
        x.reshape(8, P, D).transpose(1, 0, 2).reshape(P, 8 * D)).astype(NPBF16)

    in_maps = []
    for core in range(NCORES):
        rows = slice(core * P, (core + 1) * P)
        qb = q32[rows]                                   # [128, B]
        bits = np.empty((P, 6 * B), dtype=np.uint16)
        for l in range(6):
            bits[:, l * B:(l + 1) * B] = ((qb >> l) & 1).astype(np.uint16)
        # lhsT planes: w3[k, d, i] = R[k, d] * x[rows][i, d]
        xiT = x[rows].T                                  # [D, 128]
        w3 = R[:, :, None] * xiT[None, :, :]             # [42, 256, 128]
        # -> [42, 2, 128(d_lo), 128(i)] -> [128(d_lo), 21(pair), 2(par), 2(c), 128(i)]
        w5 = w3.reshape(NPAIR, 2, 2, P, P).transpose(3, 0, 1, 2, 4)
        w_p = np.ascontiguousarray(w5.reshape(P, W_COLS)).astype(NPBF16)
        wx_p = np.ascontiguousarray(
            np.concatenate([w_p[:, 0:512], xt_p], axis=1))
        in_maps.append(
            {
                "wx": wx_p,
                "w": w_p,
                "x": x_p,
                "bits": bits,
            }
        )
    return in_maps


def kernel(x, x_mask, q, f, R_emb):
    in_maps = make_in_maps(x, q, R_emb)
    res = run_bass_kernel_spmd(_get_nc(), in_maps, core_ids=list(range(NCORES)))
    out = np.concatenate([res.results[c]["out"] for c in range(NCORES)], axis=0)
    return out
